# revision 34
# baseline (speedup 1.0000x reference)
"""Gumbel top-k sampler for Trainium2 (Bass/Tile), 8-core data parallel. v4.

Math (per row, vocab V=50257, k=50):
    g    = logits + noise,  noise = -ln(-ln(u + eps) + eps)
    t    ~= k-th largest of g        (threshold)
    mask = sigmoid(g - t)
    out  = softmax(logits * mask)

v4 design ("y-space"): work in the exp domain so the ACT engine runs only
two big passes (both Exp, one table set, zero table switches) and the
sigmoid becomes an exact algebraic form evaluated by one fused custom DVE
op:

    w  = -ln(u + eps)            host-side 16-bit codec for u: fp16 w is
                                 *more* accurate in the u->1 tail (which
                                 decides top-k) than a linear uint16 code
    el = exp(-l)                 [ACT Exp]
    y  = w * el = exp(-g)        [DVE tensor_tensor mult, 2x fp16]
    g > t  <=>  y < s,  s = exp(-t)
    sigmoid(g-t) = 1/(1 + e^(t-g)) = s/(s + y)            (exact identity)
    masked = l*s/(s+y)           [one 8-stage custom DVE op, see below]
    out = exp(masked)/Z          [ACT Exp + row-sum accum, PE 16-part sum]

Threshold: Newton on W=8 strided block minima of y (3 fold tensor_tensor
mins at 2x fp16 beat the 1x tensor_reduce; tensor_scalar+accum counts on
the [128,393] minima are near overhead-floor):
    s' = s / (cnt/k + 0.01),  cnt = #{block-min y < s}
    final iter damped: s' = s * exp(-0.7 * ln(cnt/k + 0.01))  [tiny ACT,
    same natural_log_exp table set -> one table load total]

MASKED_SIGMOID_ANT custom DVE op (registered at import; the documented
dve_ops authoring flow): out = Src1*C0*recip(Src0+C0), with recip =
BITWISE_NOT exponent-flip seed + one Newton-Raphson pass (max rel err
1.7e-3, measured end-to-end l2 ~1.6e-3):
    x = Src0 + C0; nx = ~x; y0 = nx*C1; y1 = y0*(C2 - x*y0)
    out = y1 * Src1 * C0         = exactly 8 ALU stages

Engine budget per [128x3144] tile: ACT 2x2860ns; DVE y 1637 + 3 counts
~2760 + sigma 3375 + zscale 919 ~= 8.9us. DMA 77MB/core ~215us roofline.
"""

import numpy as np

import concourse.bass as bass
import concourse.bacc as bacc
import concourse.tile as tile
from concourse import mybir
from concourse import dve_ops as _dve_ops
from concourse.bass_utils import run_bass_kernel_spmd
from concourse.dve_spec import AluOp, Bin, C0, C1, C2, Spec, Src0, Src1, _has_src1, lower
from concourse.dve_uop import DveOpSpec

F32 = mybir.dt.float32
F16 = mybir.dt.float16
AF = mybir.ActivationFunctionType
ALU = mybir.AluOpType

B, V = 2048, 50257
NCORES = 8
ROWS = B // NCORES            # 256 rows per core
TOK = 8                       # rows per tile
NPART = 128
VPAD = 50304                  # 16 * 3144
CHUNK = VPAD // 16            # 3144 elements per partition
NTILES = ROWS // TOK          # 32 tiles per core
GROUP = 4                     # tiles per pipeline group

EPS = 1e-10
NEWTON_ITERS = 3
LNBIAS = 0.01                 # count clamp: f = cnt/k + 0.01 bounds the step
NEWTON_T0 = 5.5               # init threshold (randn logits + gumbel, k=50)
S0 = float(np.exp(-NEWTON_T0))
DAMP_LAST = 0.7               # damping on final Newton step
OUT_SCALE = 1024.0            # fp16 output scale (keeps probs in normal range)
W_MIN = 6e-6                  # w clip: keeps y in (sub)normal fp16, noise<=12

# Chebyshev-minimax seed pair for x*bitcast(~x) in [-4.5, -4] (see
# dve_ops.RECIP_APPROX_FAST_CONSTS; same seed, one NR pass instead of two)
RECIP_C1 = -0.23549792
RECIP_C2 = 2.0017324

# pads: l=0, w=1 -> y = 1*exp(0) = 1 (never below s~4e-3 so never counted);
# masked = 0*s/(s+1) = 0 exactly so each pad adds exp(0)=1 to its row's
# softmax sum; subtract NPADS from Z.
NPADS = VPAD - V              # 47


def _ref_masked_sigmoid(in0, in1, s0, s1, imm2):
    x = in0.astype(np.float32) + np.float32(s0)
    nx = (~x.view(np.int32)).view(np.float32)
    y0 = nx * np.float32(s1)
    y1 = y0 * (np.float32(imm2) - x * y0)
    return (y1 * in1.astype(np.float32) * np.float32(s0)).astype(np.float32)


def _register_masked_sigmoid():
    """out = Src1*C0/(Src0+C0) via ~x seed + 1 NR (8 ALU stages)."""
    name = "MASKED_SIGMOID_ANT"
    for op in _dve_ops.OPS:
        if op.name == name:
            return op
    x = Src0 + C0
    nx = Bin(AluOp.BITWISE_NOT, x, x)
    y0 = nx * C1
    y1 = y0 * (C2 - x * y0)
    spec = Spec(body=y1 * Src1 * C0, reference=_ref_masked_sigmoid)
    row = _dve_ops._CUSTOM_DVE_ROW_BASE + len(_dve_ops.OPS)
    shas = {}
    for ver in ("v3", "v4"):
        uops = lower(spec, ver=ver)
        shas[ver] = DveOpSpec(
            name=name, opcode=row, uops=uops, rd1_en=_has_src1(spec)
        ).sha(ver)
    op = _dve_ops.DveOp(name, spec, subdim=False, uops_sha=shas)
    _dve_ops.OPS.append(op)
    _dve_ops.CUSTOM_DVE_SPECS[name] = spec
    _dve_ops._SUB_OPCODE_FOR_NAME[name] = row
    return op


MASKED_SIGMOID = _register_masked_sigmoid()

H1, H2, H3 = 1572, 786, 393   # fold-tree sizes (3144 -> 393, W=8 strided)
NBLK = H3
COUNT_ON_POOL = False         # Pool rejects TensorScalar (ISA ucode-only)


def _build_program(k: int, ntiles: int = NTILES):
    assert 1 <= k <= 1000
    nc = bacc.Bacc("TRN2", target_bir_lowering=False, debug=False)

    # activation float biases must exist as [128,1] const APs in SBUF
    for cval in (LNBIAS,):
        ct = nc.alloc_sbuf_tensor(f"const-float32-{cval}", [128, 1], F32)
        nc.gpsimd.memset(ct.ap(), cval)
        nc.const_aps.aps[(F32, cval)] = ct.ap()
    nc.all_engine_barrier()

    # Pre-place one activation-table load for the single set that contains
    # every function used (Exp + Ln). Without this the fixpoint pass picks
    # the first set containing each function (exp_and_others / natural_log)
    # and ping-pongs ~2 loads per group.
    from concourse.hw_specs import get_activation_tables
    _tables = list(get_activation_tables(nc.m.arch))
    _setid = _tables.index("natural_log_exp_and_others")
    nc.scalar.add_instruction(mybir.InstLoadActFuncSet(
        name=nc.get_next_instruction_name(), act_func_set_id=_setid))

    l_dram = nc.dram_tensor("logits", [ntiles * TOK * VPAD], F16,
                            kind="ExternalInput")
    w_dram = nc.dram_tensor("w", [ntiles * TOK * VPAD], F16,
                            kind="ExternalInput")
    # 16x16 block-diagonal ones: row-sum + broadcast over each row's 16
    # partitions in one matmul
    m16_dram = nc.dram_tensor("m16", [NPART, NPART], F32, kind="ExternalInput")
    o_dram = nc.dram_tensor("out", [ntiles * TOK, VPAD], F16,
                            kind="ExternalOutput")

    from contextlib import ExitStack
    with tile.TileContext(nc) as tc, ExitStack() as es:
        consts = es.enter_context(tc.tile_pool(name="consts", bufs=1))
        # lt lives load(s) .. store+out-dma(s+3): 4 groups in flight, +2
        # slack so loads don't serialize on the oldest group's out-DMAs
        lpool = es.enter_context(tc.tile_pool(name="lpool", bufs=3 * GROUP + 1))
        wpool = es.enter_context(tc.tile_pool(name="wpool", bufs=3 * GROUP + 1))
        elpool = es.enter_context(tc.tile_pool(name="elpool", bufs=3))
        m12pool = es.enter_context(tc.tile_pool(name="m12pool", bufs=2))
        mxpool = es.enter_context(tc.tile_pool(name="mxpool", bufs=2 * GROUP + 2))
        cscpool = es.enter_context(tc.tile_pool(name="cscpool", bufs=4))
        gsm = es.enter_context(tc.tile_pool(name="gsm", bufs=16))
        psum = es.enter_context(tc.tile_pool(name="psum", bufs=4, space="PSUM"))

        m16 = consts.tile([NPART, NPART], F32, tag="m16")
        nc.sync.dma_start(m16[:], m16_dram.ap())
        s0g = consts.tile([NPART, GROUP], F32, tag="s0g")
        nc.vector.memset(s0g[:], S0)

        def in_ap(handle, i):
            # contiguous [128 partitions, 3144] view of padded rows 8i..8i+7
            return bass.AP(handle, i * TOK * VPAD,
                           [[CHUNK, NPART], [1, CHUNK]])

        state = {}
        gstate = {}

        def p1_ld(i):
            """load logits + w (DMA only)."""
            lt = lpool.tile([NPART, CHUNK], F16, tag="lt")
            wt = wpool.tile([NPART, CHUNK], F16, tag="wt")
            nc.sync.dma_start(lt[:], in_ap(l_dram, i))
            nc.sync.dma_start(wt[:], in_ap(w_dram, i))
            state[i] = {"lt": lt, "wt": wt}

        def p1_el(i):
            """el = exp(-l)  [ACT]"""
            el = elpool.tile([NPART, CHUNK], F16, tag="el")
            nc.scalar.activation(el[:], state[i]["lt"][:], AF.Exp, scale=-1.0)
            state[i]["el"] = el

        def p1_y(i):
            """y = w * el = exp(-g), in place over w  [DVE 2x]"""
            st_ = state[i]
            nc.vector.tensor_mul(st_["wt"][:], st_["wt"][:],
                                 st_.pop("el")[:])

        def p1_tree(i):
            """W=8 strided block minima of y via 3 fold mins  [DVE 2x]"""
            st_ = state[i]
            y = st_["wt"][:]
            m12 = m12pool.tile([NPART, H1 + H2], F16, tag="m12")
            m1 = m12[:, :H1]
            m2 = m12[:, H1:H1 + H2]
            mx = mxpool.tile([NPART, NBLK], F16, tag="mx")
            nc.vector.tensor_tensor(m1, y[:, :H1], y[:, H1:2 * H1], ALU.min)
            nc.vector.tensor_tensor(m2, m12[:, :H2], m12[:, H2:2 * H2],
                                    ALU.min)
            nc.vector.tensor_tensor(mx[:], m2[:, :H3], m2[:, H3:2 * H3],
                                    ALU.min)
            st_["mx"] = mx

        def p2_count(gi, grp, it):
            """batched Newton count for a group: cnt_j = #{mx_j < s_j}
            [DVE]; the elementwise 0/1 output goes to a scratch buffer."""
            gs = gstate[gi]
            G = len(grp)
            cng = gsm.tile([NPART, G], F32, tag="cng")
            for j, i in enumerate(grp):
                s_ap = s0g[:, j:j + 1] if it == 0 else gs["s"][:, j:j + 1]
                csc = cscpool.tile([NPART, NBLK], F16, tag="csc")
                nc.vector.tensor_scalar(csc[:], state[i]["mx"][:], s_ap,
                                        None, ALU.is_lt, op1=ALU.add,
                                        accum_out=cng[:, j:j + 1])
            c16 = psum.tile([NPART, G], F32, tag="c16")
            nc.tensor.matmul(c16[:], m16[:], cng[:], start=True, stop=True)
            gs["c16"] = c16

        def p2_update(gi, grp, it):
            """s' = s / (cnt/k + 0.01); final iter damped via tiny Ln/Exp."""
            gs = gstate[gi]
            G = len(grp)
            c16 = gs.pop("c16")
            sn = gsm.tile([NPART, G], F32, tag="sn")
            if it < NEWTON_ITERS - 1:
                f = gsm.tile([NPART, G], F32, tag="f")
                nc.vector.tensor_scalar(f[:], c16[:], 1.0 / k, LNBIAS,
                                        ALU.mult, op1=ALU.add)
                rf = gsm.tile([NPART, G], F32, tag="rf")
                nc.vector.reciprocal(rf[:], f[:])
                sprev = s0g[:, :G] if it == 0 else gs["s"][:]
                nc.vector.tensor_mul(sn[:], sprev, rf[:])
            else:
                # s_fin = s * exp(-damp * ln(cnt/k + 0.01))  [tiny ACT x2,
                # same natural_log_exp table set as the big Exp passes]
                lf = gsm.tile([NPART, G], F32, tag="lf")
                nc.scalar.activation(lf[:], c16[:], AF.Ln, bias=LNBIAS,
                                     scale=1.0 / k)
                stp = gsm.tile([NPART, G], F32, tag="stp")
                nc.scalar.activation(stp[:], lf[:], AF.Exp, scale=-DAMP_LAST)
                nc.vector.tensor_mul(sn[:], gs["s"][:], stp[:])
            gs["s"] = sn

        def p3_mask(gi, grp):
            """masked = l*s/(s+y), one fused custom DVE op, out over l."""
            gs = gstate[gi]
            for j, i in enumerate(grp):
                st_ = state[i]
                nc.vector._custom_dve(
                    MASKED_SIGMOID, out=st_["lt"][:], in0=st_["wt"][:],
                    in1=st_["lt"][:], s0=gs["s"][:, j:j + 1], s1=RECIP_C1,
                    imm2=RECIP_C2)

        def p3_exp(gi, grp):
            """exp + accums; the 16-part Z matmul is emitted per half-group
            right after its accums land so the normalize chain for tiles
            0-1 never waits on exp(3)."""
            gs = gstate[gi]
            G = len(grp)
            sumg = gsm.tile([NPART, G], F32, tag="sumg")
            gs["z16"] = []
            h = (G + 1) // 2
            for j, i in enumerate(grp):
                st_ = state[i]
                nc.scalar.activation(st_["lt"][:], st_["lt"][:], AF.Exp,
                                     accum_out=sumg[:, j:j + 1])
                if j == h - 1 or j == G - 1:
                    lo = 0 if j == h - 1 else h
                    z16 = psum.tile([NPART, j + 1 - lo], F32, tag="z16")
                    nc.tensor.matmul(z16[:], m16[:], sumg[:, lo:j + 1],
                                     start=True, stop=True)
                    gs["z16"].append((lo, z16))

        def p3_out_half(gi, grp, hh):
            gs = gstate[gi]
            lo, z16 = gs["z16"][hh]
            G = z16.shape[1]
            zc = gsm.tile([NPART, G], F32, tag="zc")
            # Z = z16 - NPADS, pre-divided by OUT_SCALE so recip gives 1024/Z
            nc.vector.tensor_scalar(zc[:], z16[:], -float(NPADS),
                                    1.0 / OUT_SCALE, ALU.add, op1=ALU.mult)
            rz = gsm.tile([NPART, G], F32, tag="rz")
            nc.vector.reciprocal(rz[:], zc[:])
            for j in range(lo, lo + G):
                i = grp[j]
                st_ = state.pop(i)
                nc.vector.tensor_scalar_mul(st_["lt"][:], st_["lt"][:],
                                            rz[:, j - lo:j - lo + 1])
                out_view = o_dram.ap()[i * TOK:(i + 1) * TOK, :].rearrange(
                    "r (c e) -> r c e", e=CHUNK)
                nc.sync.dma_start(out_view, st_["lt"][:])

        groups = [list(range(g, min(g + GROUP, ntiles)))
                  for g in range(0, ntiles, GROUP)]
        if len(groups[-1]) == GROUP and GROUP >= 4:
            # split the last group so the pipeline drain exposes less work
            tail = groups.pop()
            h = GROUP // 2
            groups += [tail[:h], tail[h:]]

        # 3-deep software pipeline: step s runs load/el/y/tree(s) | newton
        # iters 0-1 of s-1, final iter of s-2 | mask/exp/out of s-2.
        def emit_step(s, ngroups):
            grp = groups[s] if s < ngroups else None
            pa = s - 1 if 0 <= s - 1 < ngroups else None   # newton iters 0-1
            pb = s - 2 if 0 <= s - 2 < ngroups else None   # final iter + out

            def el(j):
                if grp is not None and j < len(grp):
                    p1_el(grp[j])

            def y(j):
                if grp is not None and j < len(grp):
                    p1_y(grp[j])

            def tr(j):
                if grp is not None and j < len(grp):
                    p1_tree(grp[j])

            if grp is not None:
                gstate[s] = {}
                for i in grp:
                    p1_ld(i)
            if pb is not None:
                p2_count(pb, groups[pb], 2)
            if pa is not None:
                p2_count(pa, groups[pa], 0)
            el(0)
            el(1)
            if pb is not None:
                p2_update(pb, groups[pb], 2)   # tiny ACT Ln/Exp + mult
            y(0)
            if pa is not None:
                p2_update(pa, groups[pa], 0)
            tr(0)
            el(2)
            y(1)
            tr(1)
            if pb is not None:
                p3_mask(pb, groups[pb])
            if pa is not None:
                p2_count(pa, groups[pa], 1)
            el(3)
            y(2)
            tr(2)
            if pb is not None:
                p3_exp(pb, groups[pb])
            if pa is not None:
                p2_update(pa, groups[pa], 1)
            if pb is not None:
                p3_out_half(pb, groups[pb], 0)
            y(3)
            tr(3)
            if pb is not None and len(gstate[pb]["z16"]) > 1:
                p3_out_half(pb, groups[pb], 1)

        ng = len(groups)
        for s in range(ng + 2):
            emit_step(s, ng)

    nc.compile()
    return nc


def _m16():
    m16 = np.zeros((NPART, NPART), np.float32)
    for p in range(NPART):
        g = (p // 16) * 16
        m16[g:g + 16, p] = 1.0
    return m16


def _core_inputs(l16, w16, c):
    sl = slice(c * ROWS, (c + 1) * ROWS)
    lp = np.zeros((ROWS, VPAD), np.float16)
    lp[:, :V] = l16[sl]
    wp = np.ones((ROWS, VPAD), np.float16)
    wp[:, :V] = w16[sl]
    return {"logits": lp.reshape(-1), "w": wp.reshape(-1), "m16": _m16()}


_PROGRAM_CACHE = {}


def _program(k: int):
    if k not in _PROGRAM_CACHE:
        _PROGRAM_CACHE[k] = _build_program(k)
    return _PROGRAM_CACHE[k]


def _ensure_ntff_hook():
    """This image's antenv lacks axon_hooks; recreate it with the boot
    script's ctypes NTFF hook so trace=True works."""
    import sys
    import types
    try:
        import antenv.axon_hooks  # noqa: F401
        return
    except ImportError:
        pass
    import antenv
    mod = types.ModuleType("antenv.axon_hooks")
    _h = [None]
    mod.set_axon_ntff_profile_hook = lambda hook: _h.__setitem__(0, hook)
    mod.get_axon_ntff_profile_hook = lambda: _h[0]
    sys.modules["antenv.axon_hooks"] = mod
    antenv.axon_hooks = mod
    try:
        from trn_agent_boot.trn_boot import _ntff_profile_via_ctypes
        mod.set_axon_ntff_profile_hook(
            _ntff_profile_via_ctypes("/opt/axon/libaxon_pjrt.so"))
    except Exception:
        pass


def kernel(logits: np.ndarray, u: np.ndarray, k, _trace: bool = False):
    k = int(np.asarray(k))
    if _trace:
        _ensure_ntff_hook()
    logits = np.ascontiguousarray(logits, dtype=np.float32)
    u = np.ascontiguousarray(u, dtype=np.float32)
    assert logits.shape == (B, V) and u.shape == (B, V)

    l16 = logits.astype(np.float16)
    # 16-bit codec for u: w = -ln(u+eps) in fp16 (log-spaced in u; far more
    # accurate in the u->1 tail, which decides top-k, than linear uint16)
    w16 = np.maximum(-np.log(u + EPS), W_MIN).astype(np.float16)

    nc = _program(k)
    in_maps = [_core_inputs(l16, w16, c) for c in range(NCORES)]

    res = run_bass_kernel_spmd(nc, in_maps, core_ids=list(range(NCORES)),
                               trace=_trace)
    out = np.empty((B, V), np.float32)
    inv = 1.0 / OUT_SCALE
    for c in range(NCORES):
        out[c * ROWS:(c + 1) * ROWS] = (
            res.results[c]["out"][:, :V].astype(np.float32) * inv)
    if _trace:
        return out, res
    return out
